# revision 1
# baseline (speedup 1.0000x reference)
"""DeBERTa-MoE classifier on 8 TRN2 NeuronCores (Bass/Tile), v3.

Algorithm as baseline (single NEFF, SPMD over 8 cores): data-parallel
mean-pool + original head, AllGather pooled m (bf16), expert-parallel
top-4 MoE with one-hot gather/scatter matmuls and We2@Wp folded,
ReduceScatter, final LN classifier.

v6 over v4:
  - 4-way H-split pool/AllGather pipeline (single hsh input, column-
    sliced DMA): the last AG's exposure after the pool shrinks further
  - 1/S folded into the pool identity; pool psum->sbuf copies on DVE,
    so the Activation queue is pure DMA until phase 2 and the phase-2
    weights stream during the pool
  - m_full loads on the SP queue (free after the pool stream)
  - deeper MLP pipelining (sbB bufs=3)

v4 over v3:
  - hidden_states passed as two H-halves; the pool+AllGather pipeline
    by half, so AG(half0) overlaps the half1 DMA stream (AG exposure
    ~71us -> ~25us in the cost model)
  - weight DMAs issued on the Activation HW-DGE queue so they never
    contend with the hidden-state stream on the SP queue
  - phase-2 engine rebalance: PSUM->SBUF gather copies on Activation,
    LN affine (gamma/beta) on the Pool engine, h1T copies split
    DVE/Activation -- DVE was the phase-2 bottleneck
  - pool DMA chunk SS=8; wlog/SgT stay f32 (bf16 weighted
    logits cost ~1.2e-2 rel err through the 6-wide final LN)

Self-contained: hardcodes all shapes from the problem spec.
"""

import numpy as np
import ml_dtypes

import concourse.tile as tile
from concourse import bacc, mybir
from concourse.bass_utils import run_bass_kernel_spmd
from concourse.masks import make_identity

BF16 = ml_dtypes.bfloat16

B, S, H = 1024, 128, 1024
E, HID, TOPK, C = 16, 1024, 4, 3
EPS = 1e-5
NCORES = 8
TPC = B // NCORES          # tokens per core = 128
EPC = E // NCORES          # experts per core = 2
CAP = 384                  # per-expert token capacity (3 chunks of 128)
SLOTS = EPC * CAP          # 768 slots per core
JCH = SLOTS // 128         # 6 slot chunks
P = 128

_CACHE = {}

_BF_SIZES = [
    ("We1", P * EPC * 8 * HID),   # [hp, e*hc, f]
    ("Sg", P * 8 * SLOTS),        # [tp, tc, j]
]
_BF_OFS = {}
_o = 0
for _n, _s in _BF_SIZES:
    _BF_OFS[_n] = _o
    _o += _s
NBF = _o

_F32_SIZES = [
    ("clsT", P * 8 * TPC),        # [hp, hc, t]
    ("Wd", H * H),                # [hc, p, h] chunk-major
    ("SgT", P * JCH * B),         # [jp, jc, t]
    ("bd", H),
    ("Wo", P * 8 * C),            # [hp, hc, c]
    ("bo", C),
    ("be1", EPC * HID),
    ("g1", EPC * HID),
    ("beta1", EPC * HID),
    ("W2p", P * EPC * 8 * C),     # [hp, e, hc, c]
    ("b2p", EPC * C),
    ("wsl", P * JCH),             # [p, jch]
    ("Wf1", 2 * C * C),
    ("bf1", C),
    ("gf", C),
    ("betaf", C),
    ("Wf2", C * C),
    ("bf2", C),
]
_F32_OFS = {}
_o = 0
for _n, _s in _F32_SIZES:
    _F32_OFS[_n] = _o
    _o += _s
NF32 = _o


def _ap_shaped(base, ofs, shape):
    n = int(np.prod(shape))
    ap = base.ap()[ofs:ofs + n]
    if len(shape) == 1:
        return ap
    pat = "(" + " ".join(f"d{i}" for i in range(len(shape))) + ") -> " + \
          " ".join(f"d{i}" for i in range(len(shape)))
    kw = {f"d{i}": shape[i] for i in range(len(shape) - 1)}
    return ap.rearrange(pat, **kw)


def _build():
    dt = mybir.dt
    nc = bacc.Bacc("TRN2", target_bir_lowering=False, debug=False,
                   num_devices=NCORES)

    hsh = nc.dram_tensor("hsh", [TPC, S, H], dt.bfloat16,
                         kind="ExternalInput")
    wb = nc.dram_tensor("wb", [NBF], dt.bfloat16, kind="ExternalInput")
    wf = nc.dram_tensor("wf", [NF32], dt.float32, kind="ExternalInput")
    out_ext = nc.dram_tensor("out", [TPC, C], dt.float32, kind="ExternalOutput")

    def wfp(name, shape):
        return _ap_shaped(wf, _F32_OFS[name], shape)

    def wbp(name, shape):
        return _ap_shaped(wb, _BF_OFS[name], shape)

    rg = [list(range(NCORES))]

    with tile.TileContext(nc) as tc:
        with (
            tc.tile_pool(name="cst", bufs=1) as cst,
            tc.tile_pool(name="dram", bufs=1, space="DRAM") as dram,
        ):
            ident_b = cst.tile([P, P], dt.bfloat16)
            make_identity(nc, ident_b)
            ident_s = cst.tile([P, P], dt.bfloat16)
            nc.scalar.activation(out=ident_s, in_=ident_b,
                                 func=mybir.ActivationFunctionType.Copy,
                                 scale=1.0 / S)
            ident_f = cst.tile([P, P], dt.float32)
            make_identity(nc, ident_f)
            eps_sb = cst.tile([P, 1], dt.float32)
            nc.vector.memset(eps_sb, EPS)

            NQ = 4
            QW = H // NQ
            ag_ins = [dram.tile([TPC, QW], dt.bfloat16, name=f"ag_in{q}")
                      for q in range(NQ)]
            ag_outs = [dram.tile([B, QW], dt.bfloat16, addr_space="Shared",
                                 name=f"ag_out{q}") for q in range(NQ)]
            rs_in = dram.tile([B, C], dt.float32)
            rs_out = dram.tile([TPC, C], dt.float32)

            orig_c = cst.tile([P, C], dt.float32)   # original-head logits
            m_sb = cst.tile([P, H], dt.bfloat16)    # pooled m for my tokens

            # ---- head-weight prefetch (first in DMA queue order so the
            # head matmuls can run inside the pool's DMA-bound window) ----
            clsT_sb = cst.tile([P, 8, TPC], dt.float32)
            nc.scalar.dma_start(out=clsT_sb, in_=wfp("clsT", (P, 8, TPC)))
            Wd_sb = cst.tile([P, 8, H], dt.float32)
            nc.scalar.dma_start(out=Wd_sb, in_=wfp("Wd", (8, P, H))
                              .rearrange("a p h -> p a h"))
            bd_sb = cst.tile([P, H], dt.float32)
            nc.scalar.dma_start(out=bd_sb,
                              in_=wfp("bd", (1, H)).to_broadcast((P, H)))
            Wo_sb = cst.tile([P, 8, C], dt.float32)
            nc.scalar.dma_start(out=Wo_sb, in_=wfp("Wo", (P, 8, C)))
            bo_sb = cst.tile([P, C], dt.float32)
            nc.scalar.dma_start(out=bo_sb,
                              in_=wfp("bo", (1, C)).to_broadcast((P, C)))

            # ============ Phase 1: mean pool (H quartered) ============
            SS = 8
            with (
                tc.tile_pool(name="hsp", bufs=3) as hsp,
                tc.tile_pool(name="ps1", bufs=2, space="PSUM") as ps1,
            ):
                for q in range(NQ):
                    m_ps = ps1.tile([P, QW], dt.float32)
                    for s0 in range(0, S, SS):
                        hs_t = hsp.tile([P, SS, QW], dt.bfloat16)
                        nc.sync.dma_start(
                            out=hs_t,
                            in_=hsh[:, s0:s0 + SS, q * QW:(q + 1) * QW])
                        for si in range(SS):
                            s = s0 + si
                            nc.tensor.matmul(
                                m_ps[:, :],
                                ident_s[:, :],
                                hs_t[:, si, :],
                                start=(s == 0), stop=(s == S - 1),
                            )
                    nc.vector.tensor_copy(
                        out=m_sb[:, q * QW:(q + 1) * QW],
                        in_=m_ps[:, :])
                    nc.sync.dma_start(
                        out=ag_ins[q][:, :],
                        in_=m_sb[:, q * QW:(q + 1) * QW])
                    # AllGather this quarter while later quarters stream
                    nc.gpsimd.collective_compute(
                        "AllGather", mybir.AluOpType.bypass,
                        replica_groups=rg,
                        ins=[ag_ins[q][:, :].opt()],
                        outs=[ag_outs[q][:, :].opt()],
                    )

            # ================= Phase 1b: original head =================
            with (
                tc.tile_pool(name="sb1", bufs=2) as sb1,
                tc.tile_pool(name="ps1b", bufs=1, space="PSUM") as ps1b,
                tc.tile_pool(name="psT1", bufs=2, space="PSUM") as psT1,
            ):
                og_ps0 = ps1b.tile([P, 512], dt.float32)
                og_ps1 = ps1b.tile([P, 512], dt.float32)
                og_halves = [og_ps0, og_ps1]
                for hc in range(8):
                    for nh in range(2):
                        nc.tensor.matmul(
                            og_halves[nh][:, :],
                            clsT_sb[:, hc, :],
                            Wd_sb[:, hc, nh * 512:(nh + 1) * 512],
                            start=(hc == 0), stop=(hc == 7),
                        )
                t0 = sb1.tile([P, H], dt.float32)
                for nh in range(2):
                    nc.vector.tensor_add(
                        t0[:, nh * 512:(nh + 1) * 512],
                        og_halves[nh][:, :],
                        bd_sb[:, nh * 512:(nh + 1) * 512],
                    )
                tnh = sb1.tile([P, H], dt.float32)
                nc.scalar.activation(out=tnh, in_=t0,
                                     func=mybir.ActivationFunctionType.Tanh)
                tnhT = sb1.tile([P, 8, TPC], dt.float32)
                for hc in range(8):
                    tp_ps = psT1.tile([P, P], dt.float32)
                    nc.tensor.transpose(tp_ps[:, :],
                                        tnh[:, hc * P:(hc + 1) * P], ident_f[:, :])
                    nc.vector.tensor_copy(out=tnhT[:, hc, :], in_=tp_ps)
                og2_ps = ps1b.tile([P, C], dt.float32)
                for hc in range(8):
                    nc.tensor.matmul(og2_ps[:, :], tnhT[:, hc, :], Wo_sb[:, hc, :],
                                     start=(hc == 0), stop=(hc == 7))
                nc.vector.tensor_add(orig_c[:, :], og2_ps[:, :], bo_sb[:, :])

            # ---- phase-2 weight prefetch: issued before the AllGather so
            # these stream during it (not blocked behind m_full) ----
            with tc.tile_pool(name="p2c", bufs=1) as p2c:
                Sg_sb = p2c.tile([P, 8, SLOTS], dt.bfloat16)
                nc.scalar.dma_start(out=Sg_sb, in_=wbp("Sg", (P, 8, SLOTS)))
                SgT_sb = p2c.tile([P, JCH, B], dt.float32)
                nc.scalar.dma_start(out=SgT_sb, in_=wfp("SgT", (P, JCH, B)))
                We1_sb = p2c.tile([P, EPC * 8, HID], dt.bfloat16)
                nc.scalar.dma_start(out=We1_sb, in_=wbp("We1", (P, EPC * 8, HID)))
                W2p_sb = p2c.tile([P, EPC, 8, C], dt.float32)
                nc.scalar.dma_start(out=W2p_sb, in_=wfp("W2p", (P, EPC, 8, C)))
                be1_sb = p2c.tile([P, EPC, HID], dt.float32)
                nc.scalar.dma_start(
                    out=be1_sb,
                    in_=wfp("be1", (1, EPC, HID)).to_broadcast((P, EPC, HID)))
                g1_sb = p2c.tile([P, EPC, HID], dt.float32)
                nc.scalar.dma_start(
                    out=g1_sb,
                    in_=wfp("g1", (1, EPC, HID)).to_broadcast((P, EPC, HID)))
                beta1_sb = p2c.tile([P, EPC, HID], dt.float32)
                nc.scalar.dma_start(
                    out=beta1_sb,
                    in_=wfp("beta1", (1, EPC, HID)).to_broadcast((P, EPC, HID)))
                b2p_sb = p2c.tile([P, EPC, C], dt.float32)
                nc.scalar.dma_start(
                    out=b2p_sb,
                    in_=wfp("b2p", (1, EPC, C)).to_broadcast((P, EPC, C)))
                wsl_sb = p2c.tile([P, JCH], dt.float32)
                nc.scalar.dma_start(out=wsl_sb, in_=wfp("wsl", (P, JCH)))
                Wf1_sb = p2c.tile([2 * C, C], dt.float32)
                nc.scalar.dma_start(out=Wf1_sb, in_=wfp("Wf1", (2 * C, C)))
                bf1_sb = p2c.tile([P, C], dt.float32)
                nc.scalar.dma_start(
                    out=bf1_sb, in_=wfp("bf1", (1, C)).to_broadcast((P, C)))
                gf_sb = p2c.tile([P, C], dt.float32)
                nc.scalar.dma_start(
                    out=gf_sb, in_=wfp("gf", (1, C)).to_broadcast((P, C)))
                betaf_sb = p2c.tile([P, C], dt.float32)
                nc.scalar.dma_start(
                    out=betaf_sb,
                    in_=wfp("betaf", (1, C)).to_broadcast((P, C)))
                Wf2_sb = p2c.tile([C, C], dt.float32)
                nc.scalar.dma_start(out=Wf2_sb, in_=wfp("Wf2", (C, C)))
                bf2_sb = p2c.tile([P, C], dt.float32)
                nc.scalar.dma_start(
                    out=bf2_sb, in_=wfp("bf2", (1, C)).to_broadcast((P, C)))

                m_fulls = []
                for q in range(NQ):
                    mf = p2c.tile([P, 8, QW], dt.bfloat16,
                                  name=f"m_full{q}")
                    nc.sync.dma_start(
                        out=mf,
                        in_=ag_outs[q][:, :].rearrange(
                            "(tc tp) h -> tp tc h", tp=P))
                    m_fulls.append(mf)

                mgT_sb = p2c.tile([P, 8, SLOTS], dt.bfloat16)
                wlog_sb = p2c.tile([P, JCH, C], dt.float32)

                # gather: mgT[h, j] = sum_t m[t, h] * Sg[t, j]
                with tc.tile_pool(name="psA", bufs=2, space="PSUM") as psA:
                    for hc in range(8):
                        m_half = m_fulls[hc // 2]
                        ho = (hc % 2) * P
                        for jh in range(2):
                            mgT_ps = psA.tile([P, 384], dt.float32)
                            for tcn in range(8):
                                nc.tensor.matmul(
                                    mgT_ps[:, :],
                                    m_half[:, tcn, ho:ho + P],
                                    Sg_sb[:, tcn, jh * 384:(jh + 1) * 384],
                                    start=(tcn == 0), stop=(tcn == 7),
                                )
                            nc.scalar.activation(
                                out=mgT_sb[:, hc, jh * 384:(jh + 1) * 384],
                                in_=mgT_ps,
                                func=mybir.ActivationFunctionType.Copy)

                # expert MLP per slot chunk
                with (
                    tc.tile_pool(name="psB", bufs=2, space="PSUM") as psB,
                    tc.tile_pool(name="psT2", bufs=2, space="PSUM") as psT2,
                    tc.tile_pool(name="psC", bufs=2, space="PSUM") as psC,
                    tc.tile_pool(name="sbB", bufs=3) as sbB,
                ):
                    for jc in range(JCH):
                        e = jc // 3
                        h1_ps = psB.tile([P, HID], dt.float32)
                        for nh in range(2):
                            for hc in range(8):
                                nc.tensor.matmul(
                                    h1_ps[:, nh * 512:(nh + 1) * 512],
                                    mgT_sb[:, hc, jc * P:(jc + 1) * P],
                                    We1_sb[:, e * 8 + hc, nh * 512:(nh + 1) * 512],
                                    start=(hc == 0), stop=(hc == 7),
                                )
                        t_h1 = sbB.tile([P, HID], dt.float32)
                        nc.vector.tensor_add(
                            t_h1[:, :], h1_ps[:, :],
                            be1_sb[:, e, :])
                        stats = sbB.tile([P, 2, 6], dt.float32)
                        for sg in range(2):
                            nc.vector.bn_stats(
                                out=stats[:, sg, :],
                                in_=t_h1[:, sg * 512:(sg + 1) * 512])
                        mv = sbB.tile([P, 2], dt.float32)
                        nc.vector.bn_aggr(out=mv, in_=stats)
                        nc.scalar.activation(
                            out=mv[:, 1:2], in_=mv[:, 1:2],
                            func=mybir.ActivationFunctionType.Sqrt,
                            bias=eps_sb[:, :], scale=1.0)
                        nc.vector.reciprocal(out=mv[:, 1:2], in_=mv[:, 1:2])
                        nc.vector.tensor_scalar(
                            out=t_h1[:, :], in0=t_h1[:, :],
                            scalar1=mv[:, 0:1], scalar2=mv[:, 1:2],
                            op0=mybir.AluOpType.subtract,
                            op1=mybir.AluOpType.mult)
                        nc.gpsimd.tensor_mul(
                            t_h1[:, :], t_h1[:, :],
                            g1_sb[:, e, :])
                        nc.gpsimd.tensor_add(
                            t_h1[:, :], t_h1[:, :],
                            beta1_sb[:, e, :])
                        h1g = sbB.tile([P, HID], dt.float32)
                        nc.scalar.activation(
                            out=h1g, in_=t_h1,
                            func=mybir.ActivationFunctionType.Gelu)
                        h1T = sbB.tile([P, 8, P], dt.float32)
                        for hc in range(8):
                            tp2 = psT2.tile([P, P], dt.float32)
                            nc.tensor.transpose(
                                tp2[:, :], h1g[:, hc * P:(hc + 1) * P],
                                ident_f[:, :])
                            if hc % 2 == 0:
                                nc.vector.tensor_copy(
                                    out=h1T[:, hc, :], in_=tp2)
                            else:
                                nc.scalar.activation(
                                    out=h1T[:, hc, :], in_=tp2,
                                    func=mybir.ActivationFunctionType.Copy)
                        lg_ps = psC.tile([P, C], dt.float32)
                        for hc in range(8):
                            nc.tensor.matmul(
                                lg_ps[:, :],
                                h1T[:, hc, :],
                                W2p_sb[:, e, hc, :],
                                start=(hc == 0), stop=(hc == 7),
                            )
                        t_lg = sbB.tile([P, C], dt.float32)
                        nc.vector.tensor_add(
                            t_lg[:, :], lg_ps[:, :],
                            b2p_sb[:, e, :])
                        nc.vector.tensor_scalar_mul(
                            out=wlog_sb[:, jc, :], in0=t_lg[:, :],
                            scalar1=wsl_sb[:, jc:jc + 1])

                # scatter-add: partial[t, c] = sum_j SgT[j, t]^T wlog[j, c]
                with (
                    tc.tile_pool(name="psD", bufs=2, space="PSUM") as psD,
                    tc.tile_pool(name="sbD", bufs=2) as sbD,
                ):
                    for tcn in range(8):
                        part_ps = psD.tile([P, C], dt.float32)
                        for jc in range(JCH):
                            nc.tensor.matmul(
                                part_ps[:, :],
                                SgT_sb[:, jc, tcn * P:(tcn + 1) * P],
                                wlog_sb[:, jc, :],
                                start=(jc == 0), stop=(jc == JCH - 1),
                            )
                        part_sb = sbD.tile([P, C], dt.float32)
                        nc.vector.tensor_copy(out=part_sb, in_=part_ps)
                        nc.sync.dma_start(
                            out=rs_in[tcn * P:(tcn + 1) * P, :], in_=part_sb)

                # ================= ReduceScatter =================
                nc.gpsimd.collective_compute(
                    "ReduceScatter", mybir.AluOpType.add, replica_groups=rg,
                    ins=[rs_in[:, :].opt()], outs=[rs_out[:, :].opt()],
                )

                # ================= Final classifier =================
                with (
                    tc.tile_pool(name="sbF", bufs=1) as sbF,
                    tc.tile_pool(name="psE", bufs=1, space="PSUM") as psE,
                ):
                    moe_sb = sbF.tile([P, C], dt.float32)
                    nc.sync.dma_start(out=moe_sb, in_=rs_out[:, :])
                    comb = sbF.tile([P, 2 * C], dt.float32)
                    nc.vector.tensor_copy(out=comb[:, 0:C], in_=orig_c)
                    nc.vector.tensor_copy(out=comb[:, C:2 * C], in_=moe_sb)
                    cT_ps = psE.tile([2 * C, P], dt.float32)
                    nc.tensor.transpose(cT_ps[:, :], comb[:, :], ident_f[:, :])
                    cT = sbF.tile([2 * C, P], dt.float32)
                    nc.vector.tensor_copy(out=cT, in_=cT_ps)
                    z_ps = psE.tile([P, C], dt.float32)
                    nc.tensor.matmul(z_ps[:, :], cT[:, :], Wf1_sb[:, :],
                                     start=True, stop=True)
                    z = sbF.tile([P, C], dt.float32)
                    nc.vector.tensor_add(z[:, :], z_ps[:, :], bf1_sb[:, :])
                    st3 = sbF.tile([P, 6], dt.float32)
                    nc.vector.bn_stats(out=st3, in_=z[:, :])
                    mv3 = sbF.tile([P, 2], dt.float32)
                    nc.vector.bn_aggr(out=mv3, in_=st3)
                    nc.scalar.activation(
                        out=mv3[:, 1:2], in_=mv3[:, 1:2],
                        func=mybir.ActivationFunctionType.Sqrt,
                        bias=eps_sb[:, :], scale=1.0)
                    nc.vector.reciprocal(out=mv3[:, 1:2], in_=mv3[:, 1:2])
                    nc.vector.tensor_scalar(
                        out=z[:, :], in0=z[:, :],
                        scalar1=mv3[:, 0:1], scalar2=mv3[:, 1:2],
                        op0=mybir.AluOpType.subtract, op1=mybir.AluOpType.mult)
                    nc.vector.tensor_mul(z[:, :], z[:, :], gf_sb[:, :])
                    nc.vector.tensor_add(z[:, :], z[:, :], betaf_sb[:, :])
                    nc.scalar.activation(out=z, in_=z,
                                         func=mybir.ActivationFunctionType.Relu)
                    zT_ps = psE.tile([C, P], dt.float32)
                    nc.tensor.transpose(zT_ps[:, :], z[:, :], ident_f[:, :])
                    zT = sbF.tile([C, P], dt.float32)
                    nc.vector.tensor_copy(out=zT, in_=zT_ps)
                    o_ps = psE.tile([P, C], dt.float32)
                    nc.tensor.matmul(o_ps[:, :], zT[:, :], Wf2_sb[:, :],
                                     start=True, stop=True)
                    out_sb = sbF.tile([P, C], dt.float32)
                    nc.vector.tensor_add(out_sb[:, :], o_ps[:, :], bf2_sb[:, :])
                    nc.sync.dma_start(out=out_ext[:, :], in_=out_sb)

    nc.compile()
    return nc


def _host_prep(inputs):
    f32 = np.float32
    hs = np.asarray(inputs["hidden_states"], dtype=f32)
    cls = hs[:, 0, :]

    # routing control-plane in f64 (top-4 selection margin is ~2e-4,
    # far above f32 rounding, so this matches the reference's selection)
    r = cls.astype(np.float64) @ np.asarray(inputs["Wr"], np.float64)
    r += np.asarray(inputs["br"], np.float64)
    part = np.argpartition(-r, TOPK, axis=1)[:, :TOPK]
    vals = np.take_along_axis(r, part, axis=1)
    w = np.exp(vals - vals.max(axis=1, keepdims=True))
    w /= w.sum(axis=1, keepdims=True)
    rw = np.zeros((B, E), np.float64)
    np.put_along_axis(rw, part, w, axis=1)

    We2 = np.asarray(inputs["We2"], np.float64)
    Wp = np.asarray(inputs["Wp"], np.float64)
    W2p_all = (We2 @ Wp).astype(f32)                      # [E, HID, C]
    b2p_all = (np.asarray(inputs["be2"], np.float64) @ Wp
               + np.asarray(inputs["bp"], np.float64)).astype(f32)  # [E, C]

    # Wd chunk-major [hc, p, h]
    Wd_f = np.asarray(inputs["Wd"], f32).reshape(8, P, H)
    Wo_perm = np.ascontiguousarray(
        np.asarray(inputs["Wo"], f32).reshape(8, P, C).transpose(1, 0, 2))
    We1_all = np.asarray(inputs["We1"], f32).astype(BF16)  # [E, H, HID]
    be1_all = np.asarray(inputs["be1"], f32)
    g1_all = np.asarray(inputs["g1"], f32)
    beta1_all = np.asarray(inputs["beta1"], f32)

    in_maps = []
    for c in range(NCORES):
        t0 = c * TPC
        exps = [EPC * c + i for i in range(EPC)]
        Sg = np.zeros((B, SLOTS), BF16)
        SgT = np.zeros((SLOTS, B), np.float32)
        wsl = np.zeros((SLOTS,), f32)
        for i, e in enumerate(exps):
            toks = np.nonzero(rw[:, e] != 0.0)[0]
            if len(toks) > CAP:
                raise RuntimeError(
                    f"expert {e} over capacity: {len(toks)} > {CAP}")
            js = i * CAP + np.arange(len(toks))
            Sg[toks, js] = 1
            SgT[js, toks] = 1
            wsl[js] = rw[toks, e].astype(f32)

        wbv = np.zeros((NBF,), BF16)

        def putb(name, arr):
            o = _BF_OFS[name]
            a = np.ascontiguousarray(arr).astype(BF16).ravel()
            wbv[o:o + a.size] = a

        putb("We1",
             We1_all[exps].reshape(EPC * 8, P, HID).transpose(1, 0, 2))
        putb("Sg", Sg.reshape(8, P, SLOTS).transpose(1, 0, 2))

        wfv = np.zeros((NF32,), f32)

        def put(name, arr):
            o = _F32_OFS[name]
            a = np.ascontiguousarray(arr, dtype=f32).ravel()
            wfv[o:o + a.size] = a

        clsT = cls[t0:t0 + TPC].T               # [H, TPC]
        put("clsT", clsT.reshape(8, P, TPC).transpose(1, 0, 2))
        put("Wd", Wd_f)
        put("SgT", SgT.reshape(JCH, P, B).transpose(1, 0, 2))
        put("bd", np.asarray(inputs["bd"], f32))
        put("Wo", Wo_perm)
        put("bo", np.asarray(inputs["bo"], f32))
        put("be1", be1_all[exps])
        put("g1", g1_all[exps])
        put("beta1", beta1_all[exps])
        put("W2p", W2p_all[exps].reshape(EPC, 8, P, C).transpose(2, 0, 1, 3))
        put("b2p", b2p_all[exps])
        put("wsl", wsl.reshape(JCH, P).T)
        put("Wf1", np.asarray(inputs["Wf1"], f32))
        put("bf1", np.asarray(inputs["bf1"], f32))
        put("gf", np.asarray(inputs["gf"], f32))
        put("betaf", np.asarray(inputs["betaf"], f32))
        put("Wf2", np.asarray(inputs["Wf2"], f32))
        put("bf2", np.asarray(inputs["bf2"], f32))

        in_maps.append({
            "hsh": hs[t0:t0 + TPC].astype(BF16),
            "wb": wbv,
            "wf": wfv,
        })
    return in_maps


def kernel(**inputs):
    in_maps = _host_prep(inputs)
    if "nc" not in _CACHE:
        _CACHE["nc"] = _build()
    try:
        res = run_bass_kernel_spmd(_CACHE["nc"], in_maps,
                                   core_ids=list(range(NCORES)))
    except ModuleNotFoundError:
        # BASS_TRACE set but the axon NTFF hook module is absent on this
        # client — rerun untraced
        import os
        os.environ["BASS_NEVER_TRACE"] = "1"
        res = run_bass_kernel_spmd(_CACHE["nc"], in_maps,
                                   core_ids=list(range(NCORES)))
    _CACHE["last_results"] = res
    return np.concatenate([res.results[c]["out"] for c in range(NCORES)],
                          axis=0).astype(np.float32)

